# revision 1
# baseline (speedup 1.0000x reference)
"""Trainium2 Bass/Tile kernel: AudioXMMDiT cross-attention, sharded over 8 NeuronCores.

Sharding: data-parallel over batch (2) x tensor-parallel over heads (4 groups of 8).
Each core computes, for its (batch b, heads h0..h0+7):
    q = x[b] @ Wq_c.T ; per-head RMSNorm
    k,v = context[b] @ {Wk_c,Wv_c}.T  (with the reference's cat(k,v)->(h d j)
          column interleave resolved on the host by permuting weight rows)
    out = softmax(q_n k_n^T / 8) @ v        -> out[b, :, h0*64:(h0+8)*64]

On-chip dataflow (all matmuls bf16 with fp32 PSUM accumulation):
    xT/ctxT/W*T arrive pre-transposed (contraction dim on partitions).
    Q/K/V projections in natural layout; RMSNorm in natural layout (rsqrt via
    Ln+Exp so every ACT function lives in the natural_log_exp_and_others table
    set); q,k DMA-transposed (bf16 XBAR) into head-major layout; scores^T
    computed directly via K=64 row-packed matmul pairs (tile_position from
    base partitions 0/64); exp on ACT over 2-bank [128,2,512] tiles; AV with
    exp tiles as FWL weights and v augmented by a ones column so the softmax
    denominator falls out of the same matmuls.

The per-block q pipeline (proj+norm+transpose) for block b+1 is interleaved
pair-by-pair with the attention (scores+exp+AV) of block b, keeping the
in-order PE stream free of dependency stalls (HAM stays at full clock).
"""

import os
import sys
from contextlib import ExitStack

import numpy as np

for _p in ("/opt/trn_rl_repo",):
    if os.path.isdir(_p) and _p not in sys.path:
        sys.path.insert(0, _p)

import ml_dtypes  # noqa: E402

import concourse.bacc as bacc  # noqa: E402
import concourse.tile as tile  # noqa: E402
from concourse import bass_utils, mybir  # noqa: E402
from concourse.masks import make_identity  # noqa: E402

P = 128
DIM = 2048
KC = DIM // P  # 16 contraction chunks
HK = KC // 2   # half of the contraction chunks (split-DMA granularity)
NH = 8         # heads per core
NPAIR = NH // 2
D = 64         # head dim
DA = D + 1     # + ones column (softmax denominator)
M = 512        # context length
MC = M // P    # kpos chunks
W = NH * D     # 512 output columns per core
EPS = 1e-6
SMSCALE = float(D) ** -0.5  # 1/8
NCORES = 8

BF = mybir.dt.bfloat16
F8 = mybir.dt.float8e4
EXP_DT = BF  # dtype of exp(scores) tiles (AV lhsT): f8 halves LDWEIGHTS again
F32 = mybir.dt.float32
AX = mybir.AxisListType
AF = mybir.ActivationFunctionType
MUL = mybir.AluOpType.mult
U32 = mybir.dt.uint32
SHR = mybir.AluOpType.logical_shift_right
XOR = mybir.AluOpType.bitwise_xor
SUB = mybir.AluOpType.subtract
ADD = mybir.AluOpType.add
# 0xFFFFFFFF - 0x5f3759df (so K - t == NOT(t) - this, avoiding reverse-subtract)
RSQRT_MAGIC_COMP = 0xFFFFFFFF - 0x5F3759DF

QB = 512
QCH = QB // P


def build_nc(n_q=4096):
    NQB = n_q // QB

    nc = bacc.Bacc(None, target_bir_lowering=False)

    xT = nc.dram_tensor("xT", (DIM, n_q), BF, kind="ExternalInput")
    ctxT = nc.dram_tensor("ctxT", (DIM, M), BF, kind="ExternalInput")
    wqT = nc.dram_tensor("wqT", (DIM, W), BF, kind="ExternalInput")
    wkT = nc.dram_tensor("wkT", (DIM, W), BF, kind="ExternalInput")
    wvT = nc.dram_tensor("wvT", (DIM, W), BF, kind="ExternalInput")
    out = nc.dram_tensor("out", (n_q, W), F32, kind="ExternalOutput")

    xT_r = xT[:].rearrange("(kc p) n -> p kc n", p=P)
    ctxT_r = ctxT[:].rearrange("(kc p) n -> p kc n", p=P)
    wqT_r = wqT[:].rearrange("(kc p) n -> p kc n", p=P)
    wkT_r = wkT[:].rearrange("(kc p) n -> p kc n", p=P)
    wvT_r = wvT[:].rearrange("(kc p) n -> p kc n", p=P)

    with tile.TileContext(nc) as tc, ExitStack() as es:
        consts = es.enter_context(tc.tile_pool(name="consts", bufs=1))
        stats = es.enter_context(tc.tile_pool(name="stats", bufs=3))
        # q_f32 staging shares the stats pool (tag "qf", 4 bufs)
        qpsum = es.enter_context(tc.tile_pool(name="qpsum", bufs=2, space="PSUM"))
        spsum = es.enter_context(tc.tile_pool(name="spsum", bufs=2, space="PSUM"))
        apsum = es.enter_context(tc.tile_pool(name="apsum", bufs=2, space="PSUM"))

        cst_sb = consts.tile([P, 2], F32)
        nc.vector.memset(cst_sb[:, 0:1], EPS)
        nc.vector.memset(cst_sb[:, 1:2], 0.0)
        eps_sb = cst_sb[:, 0:1]
        zero_sb = cst_sb[:, 1:2]

        wq_sb = consts.tile([P, KC, W], BF)
        kT_sb = consts.tile([P, NPAIR, M], BF)   # [pair-local 2*64, pair, kpos]
        v_sb = consts.tile([P, MC, NH, DA], BF)  # [kpos, mc, head, d + ones]
        nc.vector.memset(v_sb, 1.0)              # ones column; rest overwritten
        ident = consts.tile([P, P], F32)
        make_identity(nc, ident)

        xpool = es.enter_context(tc.tile_pool(name="xpool", bufs=6))
        qpool = es.enter_context(tc.tile_pool(name="qpool", bufs=4))
        qtpool = es.enter_context(tc.tile_pool(name="qtpool", bufs=2))
        opool = es.enter_context(tc.tile_pool(name="opool", bufs=3))
        epool = es.enter_context(tc.tile_pool(name="epool", bufs=1))
        x_tiles, qT_tiles, exp_tiles = {}, {}, {}

        def dve_rsqrt(pool, m_ss, scale, bias):
            """y = rsqrt(m_ss*scale + bias) entirely on DVE (no ACT tables):
            magic-constant seed + 2 Newton iterations, ~1e-5 rel err."""
            shp = list(m_ss.shape)
            m = pool.tile(shp, F32, tag="rsq_m")
            nc.vector.tensor_scalar(m, m_ss, scale, bias, MUL, ADD)
            y = pool.tile(shp, F32, tag="rsq_y")
            nc.vector.tensor_scalar(
                y.bitcast(U32), m.bitcast(U32), 1, 0xFFFFFFFF, SHR, XOR)
            nc.vector.tensor_scalar(
                y.bitcast(U32), y.bitcast(U32), RSQRT_MAGIC_COMP, None, SUB)
            t = pool.tile(shp, F32, tag="rsq_t")
            for _ in range(2):
                nc.vector.tensor_tensor(t, y, y, MUL)
                nc.vector.tensor_tensor(t, t, m, MUL)
                nc.vector.tensor_scalar(t, t, -0.5, 1.5, MUL, ADD)
                nc.vector.tensor_tensor(y, y, t, MUL)
            return y

        def load_x(b, engine=None):
            eng = engine or nc.gpsimd
            hs = []
            for h in range(2):
                t = xpool.tile([P, HK, QB], BF, name=f"x{b}_{h}", tag="x")
                eng.dma_start(
                    t, xT_r[:, h * HK:(h + 1) * HK, b * QB:(b + 1) * QB])
                hs.append(t)
            x_tiles[b] = hs

        qn_tiles = {}

        qps_tiles = {}

        def qproj_chunk(b, qc, ss):
            """Project q chunk; per-chunk sumsq (ACT square from PSUM)."""
            x_sb = x_tiles[b]
            qps = qpsum.tile([P, W], F32, tag="qps")
            for kc in range(KC):
                nc.tensor.matmul(
                    qps, x_sb[kc // HK][:, kc % HK, qc * P:(qc + 1) * P],
                    wq_sb[:, kc, :],
                    start=(kc == 0), stop=(kc == KC - 1))
            sq = stats.tile([P, W], F32)
            nc.scalar.activation(sq, qps, AF.Square, bias=zero_sb)
            nc.vector.reduce_sum(
                ss[:, qc, :], sq[:].rearrange("p (h d) -> p h d", h=NH),
                axis=AX.X)
            qps_tiles[(b, qc)] = qps

        def qstats_half(b, ss, half):
            """RMSNorm tail (DVE rsqrt + rescale from PSUM) for 2 chunks."""
            qcs = (2 * half, 2 * half + 1)
            rq = dve_rsqrt(stats, ss[:, qcs[0]:qcs[0] + 2, :], 1.0 / D, EPS)
            for i, qc in enumerate(qcs):
                qps = qps_tiles.pop((b, qc))
                q_n = qpool.tile([P, W], F32, name=f"qn{b}_{qc}", tag="qn")
                nc.vector.tensor_tensor(
                    q_n[:].rearrange("p (h d) -> p h d", h=NH),
                    qps[:].rearrange("p (h d) -> p h d", h=NH),
                    rq[:, i, :, None].to_broadcast([P, NH, D]),
                    MUL)
                qn_tiles[(b, qc)] = q_n

        def qtrans_chunk(b, qc):
            """PE transpose-mode: q_n chunk -> qT (via psum, cast to bf16)."""
            q_n = qn_tiles.pop((b, qc))
            tps = qpsum.tile([P, NPAIR, P], F32, tag="qps")
            for pair in range(NPAIR):
                nc.tensor.transpose(
                    tps[:, pair, :], q_n[:, pair * P:(pair + 1) * P], ident)
            nc.vector.tensor_copy(
                out=qT_tiles[b][:, :, qc * P:(qc + 1) * P], in_=tps)

        def scores_pair(b, pair):
            """scores^T for one head pair: 2x row-packed matmuls per mc chunk,
            exp over 2-bank psum tiles (both heads, uniform 1/8 scale)."""
            qT_sb = qT_tiles[b]
            exp_sb = exp_tiles[b]
            hA = 2 * pair
            for mc in range(MC):
                sps = spsum.tile([P, 2, QB], F32, tag="sps")
                nc.tensor.matmul(
                    sps[:, 0, :],
                    kT_sb[0:D, pair, mc * P:(mc + 1) * P],
                    qT_sb[0:D, pair, :],
                    start=True, stop=True)
                nc.tensor.matmul(
                    sps[:, 1, :],
                    kT_sb[D:2 * D, pair, mc * P:(mc + 1) * P],
                    qT_sb[D:2 * D, pair, :],
                    start=True, stop=True)
                nc.scalar.activation(
                    exp_sb[:, hA:hA + 2, mc, :], sps, AF.Exp,
                    bias=zero_sb, scale=SMSCALE)

        def av_chunk(b, qc):
            exp_sb = exp_tiles[b]
            if True:
                o_sb = opool.tile([P, NH, D], F32)
                for hg in range(2):
                    avps = apsum.tile([P, 4, DA], F32, tag="avps")
                    for hh in range(4):
                        h = hg * 4 + hh
                        for mc in range(MC):
                            nc.tensor.matmul(
                                avps[:, hh, :],
                                exp_sb[:, h, mc, qc * P:(qc + 1) * P],
                                v_sb[:, mc, h, :],
                                start=(mc == 0), stop=(mc == MC - 1))
                    rec = stats.tile([P, 4], F32)
                    nc.vector.reciprocal(rec, avps[:, :, D])
                    nc.vector.tensor_tensor(
                        o_sb[:, hg * 4:(hg + 1) * 4, :],
                        avps[:, :, 0:D],
                        rec[:, :, None].to_broadcast([P, 4, D]),
                        MUL)
                nc.sync.dma_start(
                    out[b * QB + qc * P: b * QB + (qc + 1) * P, :],
                    o_sb[:].rearrange("p h d -> p (h d)"))


        # ---------------- Phase 1: K/V projections -----------------------
        # Persistent phase-2 pools are created above so they cannot alias the
        # scoped phase-1 staging below; only the exp pool (created after ph1
        # closes) reuses phase-1 SBUF, and its first write happens one full
        # iteration after phase 1 completes.
        with tc.tile_pool(name="ph1", bufs=1) as ph1:
            ctx_h = [ph1.tile([P, HK, M], BF, name=f"ctx{h}") for h in range(2)]
            wk_h = [ph1.tile([P, HK, W], BF, name=f"wk{h}") for h in range(2)]
            nc.gpsimd.dma_start(ctx_h[0], ctxT_r[:, 0:HK, :])
            nc.sync.dma_start(wk_h[0], wkT_r[:, 0:HK, :])
            nc.gpsimd.dma_start(ctx_h[1], ctxT_r[:, HK:KC, :])
            nc.sync.dma_start(wk_h[1], wkT_r[:, HK:KC, :])
            nc.gpsimd.dma_start(wq_sb, wqT_r)
            k_sb = ph1.tile([P, MC, W], F32)     # normalized k, natural layout

            wv_sb = ph1.tile([P, KC, W], BF)
            nc.sync.dma_start(wv_sb, wvT_r)
            load_x(0, engine=nc.sync)
            exp_tiles[0] = epool.tile(
                [P, NH, MC, QB], BF, name="exp0", tag="exp")
            qT_tiles[0] = qtpool.tile([P, NPAIR, QB], BF, name="qT0", tag="qT")
            ss0 = stats.tile([P, QCH, NH], F32, name="ss0", tag="ss")
            kps_l = [qpsum.tile([P, W], F32, tag="qps", name=f"kps{m}")
                     for m in range(2)] + \
                    [apsum.tile([P, W], F32, tag="avps", name=f"kps{m + 2}")
                     for m in range(2)]
            for kc in range(KC):
                for mc in range(MC):
                    nc.tensor.matmul(
                        kps_l[mc],
                        ctx_h[kc // HK][:, kc % HK, mc * P:(mc + 1) * P],
                        wk_h[kc // HK][:, kc % HK, :],
                        start=(kc == 0), stop=(kc == KC - 1))
            for mc in range(MC):
                kps = kps_l[mc]
                vps = qpsum.tile([P, W], F32, tag="qps")
                for kc in range(KC):
                    nc.tensor.matmul(
                        vps, ctx_h[kc // HK][:, kc % HK, mc * P:(mc + 1) * P],
                        wv_sb[:, kc, :],
                        start=(kc == 0), stop=(kc == KC - 1))
                # v: copy-cast into augmented layout (ones column preserved)
                nc.vector.tensor_copy(
                    out=v_sb[:, mc, :, 0:D],
                    in_=vps[:].rearrange("p (h d) -> p h d", h=NH))
                # rk = exp(-0.5*ln(mean(k^2)+eps)); k_sb = k * rk
                sq = stats.tile([P, W], F32)
                nc.scalar.activation(sq, kps, AF.Square, bias=zero_sb)
                ss = stats.tile([P, NH], F32)
                nc.vector.reduce_sum(
                    ss, sq[:].rearrange("p (h d) -> p h d", h=NH), axis=AX.X)
                rk = dve_rsqrt(stats, ss, 1.0 / D, EPS)
                qproj_chunk(0, mc, ss0)
                if mc == 1:
                    qstats_half(0, ss0, 0)
                if mc == 3:
                    qstats_half(0, ss0, 1)
                nc.vector.tensor_tensor(
                    k_sb[:, mc, :].rearrange("p (h d) -> p h d", h=NH),
                    kps[:].rearrange("p (h d) -> p h d", h=NH),
                    rk[:, :, None].to_broadcast([P, NH, D]),
                    MUL)

            # PE transpose-mode (no XBAR DMA transposes anywhere in the
            # kernel: mixing XBAR and copy modes serializes the DMA fabric)
            for mc in range(MC):
                tps = qpsum.tile([P, NPAIR, P], F32, tag="qps")
                for pair in range(NPAIR):
                    nc.tensor.transpose(
                        tps[:, pair, :],
                        k_sb[:, mc, pair * P:(pair + 1) * P],
                        ident)
                nc.vector.tensor_copy(
                    out=kT_sb[:, :, mc * P:(mc + 1) * P], in_=tps)
                qtrans_chunk(0, mc)
                if mc == 0 and NQB > 1:
                    load_x(1)

        # ---------------- Phase 2: software-pipelined main loop -----------

        for i in range(1, NQB + 1):
            bq = i if i < NQB else None      # block running its q pipeline
            ba = i - 1 if i > 0 else None    # block running attention
            if bq is not None:
                if bq + 1 < NQB:
                    load_x(bq + 1)
                exp_tiles[bq] = epool.tile(
                    [P, NH, MC, QB], EXP_DT, name=f"exp{bq}", tag="exp")
                qT_tiles[bq] = qtpool.tile(
                    [P, NPAIR, QB], BF, name=f"qT{bq}", tag="qT")
                ss = stats.tile([P, QCH, NH], F32, name=f"ss{bq}", tag="ss")
            for qc in range(QCH):
                if bq is not None:
                    qproj_chunk(bq, qc, ss)
                    if qc == 1:
                        qstats_half(bq, ss, 0)
                    if qc == 3:
                        qstats_half(bq, ss, 1)
                if ba is not None:
                    scores_pair(ba, pair=qc)
            # av/transpose interleave, transposes lagging by two av chunks so
            # the DVE stats chain finishes before the PE needs q_n
            order = [("tr", 0), ("av", 0), ("tr", 1), ("av", 1), ("tr", 2),
                     ("av", 2), ("tr", 3), ("av", 3)]
            for kind, qc in order:
                if kind == "av" and ba is not None:
                    av_chunk(ba, qc)
                if kind == "tr" and bq is not None:
                    qtrans_chunk(bq, qc)
            if ba is not None:
                del x_tiles[ba], qT_tiles[ba], exp_tiles[ba]

        debug_pools = (consts, stats, xpool, qpool, qtpool, opool,
                       epool, qpsum, spsum, apsum)

    if os.environ.get("KDEBUG_POOLS"):
        for pool in debug_pools:
            try:
                print(f"POOL {pool.name}: {pool.kb_per_partition_size()} KB/part"
                      f" bufs={pool.bufs} space={pool.space}")
                for k, meta in pool.tag_meta.items():
                    print("   ", k, meta)
            except Exception as e:
                print("POOL", pool.name, "err", e)

    nc.compile()
    return nc


_NC_CACHE = {}


def _get_nc(n_q=4096):
    if n_q not in _NC_CACHE:
        _NC_CACHE[n_q] = build_nc(n_q)
    return _NC_CACHE[n_q]


def make_in_maps(x, context, Wq, Wk, Wv):
    """Host-side shard + weight permutation. Returns one input map per core."""
    bf = ml_dtypes.bfloat16
    x = np.asarray(x)
    context = np.asarray(context)
    Wkv = np.concatenate([np.asarray(Wk), np.asarray(Wv)], axis=0)  # (4096, 2048)
    # reference: cat(k,v) reshaped (h d j): head h, dim d -> row h*128 + 2d (+1 for v)
    idx = np.arange(32)[:, None] * 128 + 2 * np.arange(64)[None, :]
    Wk_eff = Wkv[idx]       # (32, 64, 2048)
    Wv_eff = Wkv[idx + 1]   # (32, 64, 2048)
    Wq_eff = np.asarray(Wq).reshape(32, 64, 2048)

    xT = [np.ascontiguousarray(x[b].T).astype(bf) for b in range(x.shape[0])]
    ctxT = [np.ascontiguousarray(context[b].T).astype(bf)
            for b in range(context.shape[0])]

    in_maps = []
    for c in range(NCORES):
        b, hg = divmod(c, 4)
        hs = slice(hg * NH, (hg + 1) * NH)
        in_maps.append({
            "xT": xT[b],
            "ctxT": ctxT[b],
            "wqT": np.ascontiguousarray(
                Wq_eff[hs].reshape(W, DIM).T).astype(bf),
            "wkT": np.ascontiguousarray(
                Wk_eff[hs].reshape(W, DIM).T).astype(bf),
            "wvT": np.ascontiguousarray(
                Wv_eff[hs].reshape(W, DIM).T).astype(bf),
        })
    return in_maps


def assemble_output(results, n_q=4096, nb=2):
    outp = np.empty((nb, n_q, DIM), np.float32)
    for c in range(NCORES):
        b, hg = divmod(c, 4)
        outp[b, :, hg * W:(hg + 1) * W] = results[c]["out"]
    return outp


def kernel(x, context, Wq, Wk, Wv, **run_kwargs):
    nc = _get_nc(x.shape[1])
    in_maps = make_in_maps(x, context, Wq, Wk, Wv)
    res = bass_utils.run_bass_kernel_spmd(
        nc, in_maps, core_ids=list(range(NCORES)), **run_kwargs)
    out = assemble_output(res.results, n_q=x.shape[1], nb=x.shape[0])
    if run_kwargs:
        kernel.last_result = res
    return out

